# revision 6
# baseline (speedup 1.0000x reference)
"""Trainium2 Bass kernel for nn_BulkHamiltonian.

Math (derived from the reference, verified numerically):
  For each Bloch wavevector k = (kx, ky):
    phase1 = sqrt(3)*kx ;  phase2 = (sqrt3/2)*(kx + sqrt3*ky)
  With r11+r22+r33 = 1.5*I and M^-1 = [[0,I],[I,0]] (a row swap), the
  output H[b] (8x8 complex64) is:
    rows 0-3:  [0 | I4]          -- k-INDEPENDENT constant
    rows 4-7:  [L11[b] | L12]    -- k-dependent only in 16 of 64 floats
  and those 16 floats are (copies/negations of) six affine combinations
  of just FOUR transcendentals: s1=sin(ph1), c1=cos(ph1), s2=sin(ph2),
  c2=cos(ph2):
    -P00 = -0.75 - 0.75*c1          Q00 = 0.75*s1
    -P01 = (sqrt3/4)*(c1 - 1)       Q01 = -(sqrt3/4)*s1
    -P11 = -0.25 - 0.25*c1 - c2     Q11 = 0.25*s1 + s2

Kernel strategy (pure data parallel, 8 cores x 125000 elements):
  - The device computes the four per-element transcendentals and writes
    them as fp16 (8 B/element instead of 256 B for the full 8x8
    complex64 row).  The host unshard step upcasts and splices them
    through the constant lattice affine map into the final array
    (extending the baseline, which already filled the constant half of
    the output host-side).  fp16 costs ~4e-4 relative error (gate 2e-2).
  - Phase math in FRACTIONAL phase space: t = k . w / 2pi; sin/cos args
    are frac(t) and frac(t + 1/4), via two custom single-uop DVE ops
    (magic-number round fused with the affine):
      FRAC_SHIFT_ANT(x; s0,s1):      frac(x*s0 + s1)
      FRAC_COMBINE2_ANT(x,y; s0,s1): frac(x*s0 + y*s1)
    THREE DVE ops per tile: s1arg, s2arg (packed adjacent), then one
    double-width FRAC_SHIFT over [s1arg|s2arg] + 1/4 -> [c1arg|c2arg].
    All four args share Sin scale 2*pi, so ONE quad-width ACT Sin per
    tile computes s1,s2,c1,c2 f32->fp16 STRAIGHT into the DMA buffer.
  - Element mapping is column-major per core (elem = p*977 + col, padded
    to 128*977): ONE k-load DMA up front, one contiguous ~2.6KB-per-
    partition store per tile.  The last tile's Sin+store is split in two
    so the final DMA drain is half as deep.
"""

import sys
import types

import numpy as np

import concourse.bacc as bacc
import concourse.mybir as mybir
from concourse import bass_utils
from concourse import dve_ops as _dve_ops
from concourse.dve_spec import C0, C1, C2, Spec, Src0, Src1, lower as _dve_lower
from concourse.dve_uop import DveOpSpec as _DveOpSpec
from concourse.tile import TileContext


def _ensure_axon_hooks():
    """bass_utils imports antenv.axon_hooks when tracing is requested (e.g.
    BASS_TRACE=1); that module isn't shipped in this image. Provide it,
    backed by the boot helper's ctypes NTFF hook when available."""
    try:
        import antenv.axon_hooks  # noqa: F401
        return
    except ImportError:
        pass
    hook = None
    try:
        from trn_agent_boot.trn_boot import _ntff_profile_via_ctypes

        hook = _ntff_profile_via_ctypes("/opt/axon/libaxon_pjrt.so")
    except Exception:
        hook = None
    mod = types.ModuleType("antenv.axon_hooks")
    mod.get_axon_ntff_profile_hook = lambda: hook
    mod.set_axon_ntff_profile_hook = lambda h: None
    try:
        import antenv

        sys.modules["antenv.axon_hooks"] = mod
        antenv.axon_hooks = mod
    except ImportError:
        sys.modules["antenv.axon_hooks"] = mod


_ensure_axon_hooks()


def _register_dve_op(name, spec):
    """Register a custom DVE op into concourse.dve_ops' tables (same row
    space as the built-ins; rows [1, 0x20) with 16 used)."""
    if name in _dve_ops._SUB_OPCODE_FOR_NAME:
        return next(op for op in _dve_ops.OPS if op.name == name)
    shas = {}
    for ver in ("v3", "v4"):
        uops = _dve_lower(spec, ver=ver)
        shas[ver] = _DveOpSpec(name=name, opcode=1, uops=uops, rd1_en=False).sha(ver)
    row = _dve_ops._CUSTOM_DVE_ROW_BASE + len(_dve_ops.OPS)
    op = _dve_ops.DveOp(name, spec, False, shas)
    _dve_ops.OPS.append(op)
    _dve_ops.CUSTOM_DVE_SPECS[name] = spec
    _dve_ops._SUB_OPCODE_FOR_NAME[name] = row
    return op


def _frac_ref(t, imm2):
    m = np.float32(imm2)
    return t - ((t + m) - m)


# out = frac(in0*s0 + s1), frac via the magic-number round (imm2 = 1.5*2^23)
OP_FRAC_SHIFT = _register_dve_op(
    "FRAC_SHIFT_ANT",
    Spec(
        body=(Src0 * C0 + C1) - (((Src0 * C0 + C1) + C2) - C2),
        reference=lambda in0, in1, s0, s1, imm2: _frac_ref(
            in0 * np.float32(s0) + np.float32(s1), imm2
        ),
    ),
)
# out = frac(in0*s0 + in1*s1)
OP_FRAC_COMBINE2 = _register_dve_op(
    "FRAC_COMBINE2_ANT",
    Spec(
        body=(Src0 * C0 + Src1 * C1) - (((Src0 * C0 + Src1 * C1) + C2) - C2),
        reference=lambda in0, in1, s0, s1, imm2: _frac_ref(
            in0 * np.float32(s0) + in1 * np.float32(s1), imm2
        ),
    ),
)

B_TOTAL = 1_000_000
N_CORES = 8
N_PER_CORE = B_TOTAL // N_CORES   # 125000
NCOLS = (N_PER_CORE + 127) // 128  # 977 columns per partition (padded)
N_PAD = 128 * NCOLS                # 125056 elements incl pad
NVAL = 4                           # s1, s2, c1, c2 per element

F32 = mybir.dt.float32
F16 = mybir.dt.float16

SQ3 = 1.7320508075688772
C34 = np.float32(0.4330127018922193)   # sqrt(3)/4
PI = 3.141592653589793
MAGIC = 12582912.0                     # 1.5 * 2**23 float32 rounding trick
INV1 = SQ3 / (2 * PI)                  # phase1 = 2pi * (kx*INV1)
INV2 = SQ3 / (4 * PI)                  # phase2 = 2pi * (kx*INV2 + ky*sqrt3*INV2)
SQ3INV2 = SQ3 * INV2

# column-range tiles; the last one is split in two for a shallower drain
TILE_COLS = [(0, 326), (326, 326), (652, 163), (815, 162)]
assert sum(n for _, n in TILE_COLS) == NCOLS
SLOT = 4 * 326  # fixed DRAM slot stride per (partition, tile)

# constant parts of the rows-4..7 slab ([4,8] complex64 = [4,16] f32)
SLAB_TEMPLATE = np.zeros(64, dtype=np.float32)
for _c, _v in [(0, 1.5), (18, 1.5), (36, 1.5), (54, 1.5),
               (11, 0.2), (25, -0.2), (47, 0.2), (61, -0.2)]:
    SLAB_TEMPLATE[_c] = _v

# constant top rows 0..3 of H: [0 | I4]
TOP_CONST = np.zeros((4, 8), dtype=np.complex64)
for _rr in range(4):
    TOP_CONST[_rr, 4 + _rr] = 1.0


def build_nc(enable_asserts=False):
    nc = bacc.Bacc(
        "TRN2",
        target_bir_lowering=False,
        debug=False,
        enable_asserts=enable_asserts,
    )
    k_ap = nc.dram_tensor("k_in", [N_PAD, 2], F32, kind="ExternalInput").ap()
    nt = len(TILE_COLS)
    o_ap = nc.dram_tensor("h_out", [128, nt, SLOT], F16, kind="ExternalOutput").ap()

    k_all = nc.alloc_sbuf_tensor("k_all", [128, NCOLS, 2], F32).ap()
    obufs = [
        nc.alloc_sbuf_tensor(f"obuf{t}", [128, NVAL, nbt], F16).ap()
        for t, (_, nbt) in enumerate(TILE_COLS)
    ]

    AF = mybir.ActivationFunctionType

    with TileContext(nc) as tc:
        # single k-load: element = p*NCOLS + col
        nc.sync.dma_start(
            k_all,
            k_ap.rearrange("(p n) c -> p n c", p=128),
        )

        with tc.tile_pool(name="work", bufs=2) as pool:
            for t, (c0, nbt) in enumerate(TILE_COLS):
                kx = k_all[:, c0:c0 + nbt, 0]
                ky = k_all[:, c0:c0 + nbt, 1]
                # arg pack layout: [s1arg, s2arg, c1arg, c2arg]
                ap_ = pool.tile([128, NVAL, nbt], F32, tag="args", name=f"args{t}")

                nc.vector._custom_dve(
                    OP_FRAC_SHIFT, out=ap_[:, 0, :], in0=kx,
                    s0=INV1, s1=0.0, imm2=MAGIC)
                nc.vector._custom_dve(
                    OP_FRAC_COMBINE2, out=ap_[:, 1, :], in0=ky, in1=kx,
                    s0=SQ3INV2, s1=INV2, imm2=MAGIC)
                # [c1arg|c2arg] = frac([s1arg|s2arg] + 1/4), double-width
                nc.vector._custom_dve(
                    OP_FRAC_SHIFT, out=ap_[:, 2:4, :], in0=ap_[:, 0:2, :],
                    s0=1.0, s1=0.25, imm2=MAGIC)

                # one quad-width Sin straight into the DMA buffer
                nc.scalar.activation(obufs[t], ap_, AF.Sin, bias=0.0, scale=2 * PI)

                nc.sync.dma_start(
                    o_ap[:, t, :NVAL * nbt],
                    obufs[t].rearrange("p c n -> p (c n)"),
                )

    nc.compile()
    return nc


_CACHE = {}


def _get_nc():
    if "nc" not in _CACHE:
        _CACHE["nc"] = build_nc()
    return _CACHE["nc"]


def run_spmd(k_flat, **kwargs):
    """k_flat: [B_TOTAL, 2] float32. Returns (per-core results, res obj)."""
    k_flat = np.ascontiguousarray(k_flat).reshape(N_CORES, N_PER_CORE, 2)
    shards = np.zeros((N_CORES, N_PAD, 2), dtype=np.float32)
    shards[:, :N_PER_CORE, :] = k_flat
    nc = _get_nc()
    in_maps = [{"k_in": shards[i]} for i in range(N_CORES)]
    res = bass_utils.run_bass_kernel_spmd(
        nc, in_maps, core_ids=list(range(N_CORES)), **kwargs
    )
    return [res.results[i]["h_out"] for i in range(N_CORES)], res


def _decode_shard(raw):
    """raw: [128, ntiles, SLOT] fp16 -> [N_PER_CORE, 4] f32 in element
    order (elem = p*NCOLS + col): s1, s2, c1, c2."""
    vals = np.empty((128, NVAL, NCOLS), dtype=np.float16)
    for t, (c0, nbt) in enumerate(TILE_COLS):
        vals[:, :, c0:c0 + nbt] = raw[:, t, :NVAL * nbt].reshape(128, NVAL, nbt)
    v = vals.transpose(0, 2, 1).reshape(N_PAD, NVAL)[:N_PER_CORE]
    return v.astype(np.float32)


def kernel(k):
    k = np.asarray(k, dtype=np.float32).reshape(B_TOTAL, 2)
    shards, _ = run_spmd(k)

    H = np.empty((B_TOTAL, 8, 8), dtype=np.complex64)
    H[:, 0:4, :] = TOP_CONST  # constant [0 | I4] top rows
    Hf = H.view(np.float32).reshape(B_TOTAL, 8, 16)
    Hf[:, 4:8, :] = SLAB_TEMPLATE.reshape(4, 16)

    for i in range(N_CORES):
        v = _decode_shard(np.asarray(shards[i]))
        s1, s2, c1, c2 = v[:, 0], v[:, 1], v[:, 2], v[:, 3]
        # six distinct values of the hopping blocks
        p00n = -0.75 - 0.75 * c1          # -P00
        q00 = 0.75 * s1                   # +Q00
        p01n = C34 * c1 - C34             # -P01
        q01 = -C34 * s1                   # +Q01
        p11n = (-0.25 - 0.25 * c1) - c2   # -P11
        q11 = 0.25 * s1 + s2              # +Q11
        sl = Hf[i * N_PER_CORE:(i + 1) * N_PER_CORE]
        # splice into the rows-4..7 slab (flat float col c -> [4+c//16, c%16])
        for val, cols in [
            (p00n, (4, 32)), (q00, (5,)), (-q00, (33,)),
            (p01n, (6, 20, 34, 48)), (q01, (7, 21)), (-q01, (35, 49)),
            (p11n, (22, 50)), (q11, (23,)), (-q11, (51,)),
        ]:
            for c in cols:
                sl[:, 4 + c // 16, c % 16] = val
    return H


# revision 8
# speedup vs baseline: 1.2008x; 1.2008x over previous
"""Trainium2 Bass kernel for nn_BulkHamiltonian.

Math (derived from the reference, verified numerically):
  For each Bloch wavevector k = (kx, ky):
    phase1 = sqrt(3)*kx ;  phase2 = (sqrt3/2)*(kx + sqrt3*ky)
  With r11+r22+r33 = 1.5*I and M^-1 = [[0,I],[I,0]] (a row swap), the
  output H[b] (8x8 complex64) is:
    rows 0-3:  [0 | I4]          -- k-INDEPENDENT constant
    rows 4-7:  [L11[b] | L12]    -- k-dependent only in 16 of 64 floats
  and those 16 floats are (copies/negations of) six affine combinations
  of just FOUR transcendentals: s1=sin(ph1), c1=cos(ph1), s2=sin(ph2),
  c2=cos(ph2):
    -P00 = -0.75 - 0.75*c1          Q00 = 0.75*s1
    -P01 = (sqrt3/4)*(c1 - 1)       Q01 = -(sqrt3/4)*s1
    -P11 = -0.25 - 0.25*c1 - c2     Q11 = 0.25*s1 + s2

Kernel strategy (pure data parallel, 8 cores x 125000 elements):
  - The device computes the four per-element transcendentals and writes
    them as fp16 (8 B/element instead of 256 B for the full 8x8
    complex64 row).  The host unshard step upcasts and splices them
    through the constant lattice affine map into the final array
    (extending the baseline, which already filled the constant half of
    the output host-side).  fp16 costs ~4e-4 relative error (gate 2e-2).
  - Phase math in FRACTIONAL phase space: t = k . w / 2pi; sin/cos args
    are frac(t) and frac(t + 1/4), via two custom single-uop DVE ops
    (magic-number round fused with the affine):
      FRAC_SHIFT_ANT(x; s0,s1):      frac(x*s0 + s1)
      FRAC_COMBINE2_ANT(x,y; s0,s1): frac(x*s0 + y*s1)
    THREE DVE ops per tile: s1arg, s2arg (packed adjacent), then one
    double-width FRAC_SHIFT over [s1arg|s2arg] + 1/4 -> [c1arg|c2arg].
    All four args share Sin scale 2*pi, so ONE quad-width ACT Sin per
    tile computes s1,s2,c1,c2 f32->fp16 STRAIGHT into the DMA buffer.
  - Element mapping is column-major per core (elem = p*977 + col, padded
    to 128*977): ONE k-load DMA up front, one contiguous ~2.6KB-per-
    partition store per tile.  The last tile's Sin+store is split in two
    so the final DMA drain is half as deep.
"""

import sys
import types

import numpy as np

import concourse.bacc as bacc
import concourse.mybir as mybir
from concourse import bass_utils
from concourse import dve_ops as _dve_ops
from concourse.dve_spec import C0, C1, C2, Spec, Src0, Src1, lower as _dve_lower
from concourse.dve_uop import DveOpSpec as _DveOpSpec
from concourse.tile import TileContext


def _ensure_axon_hooks():
    """bass_utils imports antenv.axon_hooks when tracing is requested (e.g.
    BASS_TRACE=1); that module isn't shipped in this image. Provide it,
    backed by the boot helper's ctypes NTFF hook when available."""
    try:
        import antenv.axon_hooks  # noqa: F401
        return
    except ImportError:
        pass
    hook = None
    try:
        from trn_agent_boot.trn_boot import _ntff_profile_via_ctypes

        hook = _ntff_profile_via_ctypes("/opt/axon/libaxon_pjrt.so")
    except Exception:
        hook = None
    mod = types.ModuleType("antenv.axon_hooks")
    mod.get_axon_ntff_profile_hook = lambda: hook
    mod.set_axon_ntff_profile_hook = lambda h: None
    try:
        import antenv

        sys.modules["antenv.axon_hooks"] = mod
        antenv.axon_hooks = mod
    except ImportError:
        sys.modules["antenv.axon_hooks"] = mod


_ensure_axon_hooks()


def _register_dve_op(name, spec):
    """Register a custom DVE op into concourse.dve_ops' tables (same row
    space as the built-ins; rows [1, 0x20) with 16 used)."""
    if name in _dve_ops._SUB_OPCODE_FOR_NAME:
        return next(op for op in _dve_ops.OPS if op.name == name)
    shas = {}
    for ver in ("v3", "v4"):
        uops = _dve_lower(spec, ver=ver)
        shas[ver] = _DveOpSpec(name=name, opcode=1, uops=uops, rd1_en=False).sha(ver)
    row = _dve_ops._CUSTOM_DVE_ROW_BASE + len(_dve_ops.OPS)
    op = _dve_ops.DveOp(name, spec, False, shas)
    _dve_ops.OPS.append(op)
    _dve_ops.CUSTOM_DVE_SPECS[name] = spec
    _dve_ops._SUB_OPCODE_FOR_NAME[name] = row
    return op


def _frac_ref(t, imm2):
    m = np.float32(imm2)
    return t - ((t + m) - m)


# out = frac(in0*s0 + s1), frac via the magic-number round (imm2 = 1.5*2^23)
OP_FRAC_SHIFT = _register_dve_op(
    "FRAC_SHIFT_ANT",
    Spec(
        body=(Src0 * C0 + C1) - (((Src0 * C0 + C1) + C2) - C2),
        reference=lambda in0, in1, s0, s1, imm2: _frac_ref(
            in0 * np.float32(s0) + np.float32(s1), imm2
        ),
    ),
)
# out = frac(in0*s0 + in1*s1)
OP_FRAC_COMBINE2 = _register_dve_op(
    "FRAC_COMBINE2_ANT",
    Spec(
        body=(Src0 * C0 + Src1 * C1) - (((Src0 * C0 + Src1 * C1) + C2) - C2),
        reference=lambda in0, in1, s0, s1, imm2: _frac_ref(
            in0 * np.float32(s0) + in1 * np.float32(s1), imm2
        ),
    ),
)

B_TOTAL = 1_000_000
N_CORES = 8
N_PER_CORE = B_TOTAL // N_CORES   # 125000
NCOLS = (N_PER_CORE + 127) // 128  # 977 columns per partition (padded)
N_PAD = 128 * NCOLS                # 125056 elements incl pad
NVAL = 4                           # s1, s2, c1, c2 per element

F32 = mybir.dt.float32
F16 = mybir.dt.float16

SQ3 = 1.7320508075688772
C34 = np.float32(0.4330127018922193)   # sqrt(3)/4
PI = 3.141592653589793
MAGIC = 12582912.0                     # 1.5 * 2**23 float32 rounding trick
INV1 = SQ3 / (2 * PI)                  # phase1 = 2pi * (kx*INV1)
INV2 = SQ3 / (4 * PI)                  # phase2 = 2pi * (kx*INV2 + ky*sqrt3*INV2)
SQ3INV2 = SQ3 * INV2

# column-range tiles; small first tile so compute starts as soon as the
# first (cheap) k chunk lands
TILE_COLS = [(0, 132), (132, 281), (413, 282), (695, 282)]
assert sum(n for _, n in TILE_COLS) == NCOLS
SLOT = 4 * 282  # fixed DRAM slot stride per (partition, tile)

# constant parts of the rows-4..7 slab ([4,8] complex64 = [4,16] f32)
SLAB_TEMPLATE = np.zeros(64, dtype=np.float32)
for _c, _v in [(0, 1.5), (18, 1.5), (36, 1.5), (54, 1.5),
               (11, 0.2), (25, -0.2), (47, 0.2), (61, -0.2)]:
    SLAB_TEMPLATE[_c] = _v

# constant top rows 0..3 of H: [0 | I4]
TOP_CONST = np.zeros((4, 8), dtype=np.complex64)
for _rr in range(4):
    TOP_CONST[_rr, 4 + _rr] = 1.0


def build_nc(enable_asserts=False):
    nc = bacc.Bacc(
        "TRN2",
        target_bir_lowering=False,
        debug=False,
        enable_asserts=enable_asserts,
    )
    k_ap = nc.dram_tensor("k_in", [N_PAD, 2], F32, kind="ExternalInput").ap()
    nt = len(TILE_COLS)
    o_ap = nc.dram_tensor("h_out", [128, nt, SLOT], F16, kind="ExternalOutput").ap()

    k_all = nc.alloc_sbuf_tensor("k_all", [128, NCOLS, 2], F32).ap()
    obufs = [
        nc.alloc_sbuf_tensor(f"obuf{t}", [128, NVAL, nbt], F16).ap()
        for t, (_, nbt) in enumerate(TILE_COLS)
    ]

    AF = mybir.ActivationFunctionType

    with TileContext(nc) as tc:
        # per-tile k loads (element = p*NCOLS + col): tile 0's small chunk
        # lands first so compute starts early
        k_r = k_ap.rearrange("(p n) c -> p n c", p=128)
        for c0, nbt in TILE_COLS:
            nc.sync.dma_start(
                k_all[:, c0:c0 + nbt, :],
                k_r[:, c0:c0 + nbt, :],
            )

        with tc.tile_pool(name="work", bufs=2) as pool:
            for t, (c0, nbt) in enumerate(TILE_COLS):
                kx = k_all[:, c0:c0 + nbt, 0]
                ky = k_all[:, c0:c0 + nbt, 1]
                # arg pack layout: [s1arg, s2arg, c1arg, c2arg]
                ap_ = pool.tile([128, NVAL, nbt], F32, tag="args", name=f"args{t}")

                nc.vector._custom_dve(
                    OP_FRAC_SHIFT, out=ap_[:, 0, :], in0=kx,
                    s0=INV1, s1=0.0, imm2=MAGIC)
                nc.vector._custom_dve(
                    OP_FRAC_COMBINE2, out=ap_[:, 1, :], in0=ky, in1=kx,
                    s0=SQ3INV2, s1=INV2, imm2=MAGIC)
                # [c1arg|c2arg] = frac([s1arg|s2arg] + 1/4), double-width
                nc.vector._custom_dve(
                    OP_FRAC_SHIFT, out=ap_[:, 2:4, :], in0=ap_[:, 0:2, :],
                    s0=1.0, s1=0.25, imm2=MAGIC)

                # one quad-width Sin straight into the DMA buffer
                nc.scalar.activation(obufs[t], ap_, AF.Sin, bias=0.0, scale=2 * PI)

                nc.sync.dma_start(
                    o_ap[:, t, :NVAL * nbt],
                    obufs[t].rearrange("p c n -> p (c n)"),
                )

    nc.compile()
    return nc


_CACHE = {}


def _get_nc():
    if "nc" not in _CACHE:
        _CACHE["nc"] = build_nc()
    return _CACHE["nc"]


def run_spmd(k_flat, **kwargs):
    """k_flat: [B_TOTAL, 2] float32. Returns (per-core results, res obj)."""
    k_flat = np.ascontiguousarray(k_flat).reshape(N_CORES, N_PER_CORE, 2)
    shards = np.zeros((N_CORES, N_PAD, 2), dtype=np.float32)
    shards[:, :N_PER_CORE, :] = k_flat
    nc = _get_nc()
    in_maps = [{"k_in": shards[i]} for i in range(N_CORES)]
    res = bass_utils.run_bass_kernel_spmd(
        nc, in_maps, core_ids=list(range(N_CORES)), **kwargs
    )
    return [res.results[i]["h_out"] for i in range(N_CORES)], res


def _decode_shard(raw):
    """raw: [128, ntiles, SLOT] fp16 -> [N_PER_CORE, 4] f32 in element
    order (elem = p*NCOLS + col): s1, s2, c1, c2."""
    vals = np.empty((128, NVAL, NCOLS), dtype=np.float16)
    for t, (c0, nbt) in enumerate(TILE_COLS):
        vals[:, :, c0:c0 + nbt] = raw[:, t, :NVAL * nbt].reshape(128, NVAL, nbt)
    v = vals.transpose(0, 2, 1).reshape(N_PAD, NVAL)[:N_PER_CORE]
    return v.astype(np.float32)


def kernel(k):
    k = np.asarray(k, dtype=np.float32).reshape(B_TOTAL, 2)
    shards, _ = run_spmd(k)

    H = np.empty((B_TOTAL, 8, 8), dtype=np.complex64)
    H[:, 0:4, :] = TOP_CONST  # constant [0 | I4] top rows
    Hf = H.view(np.float32).reshape(B_TOTAL, 8, 16)
    Hf[:, 4:8, :] = SLAB_TEMPLATE.reshape(4, 16)

    for i in range(N_CORES):
        v = _decode_shard(np.asarray(shards[i]))
        s1, s2, c1, c2 = v[:, 0], v[:, 1], v[:, 2], v[:, 3]
        # six distinct values of the hopping blocks
        p00n = -0.75 - 0.75 * c1          # -P00
        q00 = 0.75 * s1                   # +Q00
        p01n = C34 * c1 - C34             # -P01
        q01 = -C34 * s1                   # +Q01
        p11n = (-0.25 - 0.25 * c1) - c2   # -P11
        q11 = 0.25 * s1 + s2              # +Q11
        sl = Hf[i * N_PER_CORE:(i + 1) * N_PER_CORE]
        # splice into the rows-4..7 slab (flat float col c -> [4+c//16, c%16])
        for val, cols in [
            (p00n, (4, 32)), (q00, (5,)), (-q00, (33,)),
            (p01n, (6, 20, 34, 48)), (q01, (7, 21)), (-q01, (35, 49)),
            (p11n, (22, 50)), (q11, (23,)), (-q11, (51,)),
        ]:
            for c in cols:
                sl[:, 4 + c // 16, c % 16] = val
    return H
